# revision 8
# baseline (speedup 1.0000x reference)
"""Trainium2 Bass kernel for nn_CMEncoder (cross-attention + LayerNorm2d + MLP block).

Strategy (8 NeuronCores, sequence-parallel over the HW=4096 query tokens):
  - Each core owns 512 query tokens; K/V over the full 4096-token context are
    computed redundantly on every core (no collectives needed).
  - Everything stays channel-major on chip ([feature partition, token free]).
  - Scores are computed transposed (S^T[n, q]) so P = exp(S^T) is the moving
    operand of the P@V matmuls (att^T = V^T @ P, N=512 full-rate); the softmax
    denominator comes from a cheap ones-stationary reduction matmul.
  - Host-side algebraic folds: bk dropped (softmax shift invariance), bv folded
    into the output-projection bias, the 1/sqrt(C) scale folded into the Q
    bias/scale, LayerNorm's affine folded into the MLP's first layer.
  - Matmuls run in float32r (full-rate fp32 on the PE array).
"""

import numpy as np
import concourse.bacc as bacc
import concourse.mybir as mybir
import concourse.tile as tile
from concourse import bass_utils
from concourse.hw_specs import get_activation_tables

F32 = mybir.dt.float32
F32R = mybir.dt.float32r
BF16 = mybir.dt.bfloat16
AF = mybir.ActivationFunctionType
ALU = mybir.AluOpType

MMDT = F32R      # matmul operand dtype (F32R or BF16)

C = 256          # channels
HW = 4096        # query tokens (64x64)
NCTX = 4096      # context tokens
HID = 512        # mlp hidden
NCORES = 8
QS = HW // NCORES   # 512 queries per core
NBLK = NCTX // 128  # 32 context chunks
EPS = 1e-6


def _build_nc():
    nc = bacc.Bacc("TRN2", target_bir_lowering=False)

    # --- DRAM I/O (weights pre-packed on host: row-chunks side by side) ---
    d_xmm = nc.dram_tensor("x_mm", (128, 2 * QS), MMDT, kind="ExternalInput")
    d_xf = nc.dram_tensor("x_f32", (C, QS), F32, kind="ExternalInput")
    d_y = nc.dram_tensor("y_mm", (128, 2 * NCTX), MMDT, kind="ExternalInput")
    d_wq = nc.dram_tensor("wq_mm", (128, 2 * C), MMDT, kind="ExternalInput")
    d_wk = nc.dram_tensor("wk_mm", (128, 2 * C), MMDT, kind="ExternalInput")
    d_wv = nc.dram_tensor("wv_mm", (128, 2 * C), MMDT, kind="ExternalInput")
    d_wo = nc.dram_tensor("wo_mm", (128, 2 * C), MMDT, kind="ExternalInput")
    d_w1 = nc.dram_tensor("w1_mm", (128, 2 * HID), MMDT, kind="ExternalInput")
    d_w2 = nc.dram_tensor("w2_mm", (128, 4 * C), MMDT, kind="ExternalInput")
    d_bv = nc.dram_tensor("bvec", (C, 3), F32, kind="ExternalInput")   # [bq/16, bo', b2]
    d_b1 = nc.dram_tensor("b1p", (128, 4), F32, kind="ExternalInput")
    d_oc = nc.dram_tensor("ones_c", (128, 2), MMDT, kind="ExternalInput")
    d_or = nc.dram_tensor("ones_r", (1, 128), MMDT, kind="ExternalInput")
    d_out = nc.dram_tensor("out_sh", (C, QS), F32, kind="ExternalOutput")

    tabs = list(get_activation_tables(nc.m.arch).keys())
    LNEXP_SET = tabs.index("natural_log_exp_and_others")

    with tile.TileContext(nc) as tc:
        # Pre-load the exp+ln activation table once so the auto-inserted loads
        # don't ping-pong between exp-only and ln-only sets mid-kernel.
        nc.scalar.add_instruction(mybir.InstLoadActFuncSet(
            name=nc.get_next_instruction_name(), ins=[], outs=[],
            act_func_set_id=LNEXP_SET))

        with (
            tc.tile_pool(name="sb", bufs=1) as sb,
            tc.tile_pool(name="pt_pool", bufs=3) as ptp,
            tc.tile_pool(name="ps", bufs=4, space="PSUM") as ps,
        ):
            # ---------------- input DMAs ----------------
            # compute-critical first; split issue across the Sync HW queue and
            # the GpSimd SW queue so descriptor generation isn't serialized.
            xmm = sb.tile([128, 2 * QS], MMDT)
            nc.sync.dma_start(xmm, d_xmm[:, :])
            wq_t = sb.tile([128, 2 * C], MMDT)
            nc.sync.dma_start(wq_t, d_wq[:, :])
            yt = [[sb.tile([128, NCTX // 2], MMDT, name=f"y{i}{h}") for h in range(2)]
                  for i in range(2)]
            for h in range(2):
                for i in range(2):
                    nc.sync.dma_start(
                        yt[i][h],
                        d_y[:, i * NCTX + h * (NCTX // 2):
                            i * NCTX + (h + 1) * (NCTX // 2)])

            wk_t = sb.tile([128, 2 * C], MMDT)
            nc.gpsimd.dma_start(wk_t, d_wk[:, :])
            wv_t = sb.tile([128, 2 * C], MMDT)
            nc.gpsimd.dma_start(wv_t, d_wv[:, :])
            bvec = [sb.tile([128, 3], F32, name=f"bvec{i}") for i in range(2)]
            for i in range(2):
                nc.gpsimd.dma_start(bvec[i], d_bv[i * 128:(i + 1) * 128, :])
            ones_c = sb.tile([128, 2], MMDT)
            nc.gpsimd.dma_start(ones_c, d_oc[:, :])
            ones_r = sb.tile([1, 128], MMDT)
            nc.gpsimd.dma_start(ones_r, d_or[:, :])
            wo_t = sb.tile([128, 2 * C], MMDT)
            nc.gpsimd.dma_start(wo_t, d_wo[:, :])
            w1_t = sb.tile([128, 2 * HID], MMDT)
            nc.gpsimd.dma_start(w1_t, d_w1[:, :])
            w2_t = sb.tile([128, 4 * C], MMDT)
            nc.gpsimd.dma_start(w2_t, d_w2[:, :])
            b1p = sb.tile([128, 4], F32)
            nc.gpsimd.dma_start(b1p, d_b1[:, :])
            xf = [sb.tile([128, QS], F32, name=f"xf{i}") for i in range(2)]
            for i in range(2):
                nc.gpsimd.dma_start(xf[i], d_xf[i * 128:(i + 1) * 128, :])

            epsv = sb.tile([1, 1], F32)
            nc.vector.memset(epsv, EPS)

            def wsl(t, cc, cb, w=128):
                # packed weight tile slice: row-chunk cc, col-chunk cb
                return t[:, cc * (t.shape[1] // 2) + cb * w:
                         cc * (t.shape[1] // 2) + (cb + 1) * w]

            # ---------------- Q' = (x^T Wq^T + bq)/16, channel-major ----------------
            qp = [sb.tile([128, QS], MMDT, name=f"qp{i}") for i in range(2)]
            for cb in range(2):
                qps = ps.tile([128, QS], F32, tag="work", name=f"qps{cb}")
                nc.tensor.matmul(qps, wsl(wq_t, 0, cb), xmm[:, 0:QS],
                                 start=True, stop=False)
                nc.tensor.matmul(qps, wsl(wq_t, 1, cb), xmm[:, QS:2 * QS],
                                 start=False, stop=True)
                nc.scalar.activation(qp[cb], qps, AF.Identity,
                                     bias=bvec[cb][:, 0:1], scale=1.0 / 16.0)

            # ---------------- K^T and V (token-major) ----------------
            kt = [sb.tile([128, NCTX], MMDT, name=f"kt{i}") for i in range(2)]
            v_all = sb.tile([128, NBLK * 256], MMDT)

            for nb in range(8):
                h = nb // 4
                col = (nb % 4) * 512
                for cb in range(2):
                    kps = ps.tile([128, 512], F32, tag="work", name=f"kps{cb}_{nb}")
                    nc.tensor.matmul(kps, wsl(wk_t, 0, cb),
                                     yt[0][h][:, col:col + 512], start=True, stop=False)
                    nc.tensor.matmul(kps, wsl(wk_t, 1, cb),
                                     yt[1][h][:, col:col + 512], start=False, stop=True)
                    nc.scalar.copy(kt[cb][:, nb * 512:(nb + 1) * 512], kps)
                for p2 in range(2):
                    vps = ps.tile([128, 512], F32, tag="work", name=f"vps{nb}_{p2}")
                    for k in range(2):
                        ci = nb * 4 + p2 * 2 + k   # context chunk index
                        c0 = (ci * 128) % 2048     # column within the y half-tile
                        nc.tensor.matmul(vps[:, k * 256:(k + 1) * 256],
                                         yt[0][h][:, c0:c0 + 128],
                                         wv_t[:, 0:256], start=True, stop=False)
                        nc.tensor.matmul(vps[:, k * 256:(k + 1) * 256],
                                         yt[1][h][:, c0:c0 + 128],
                                         wv_t[:, 256:512], start=False, stop=True)
                    ci0 = nb * 4 + p2 * 2
                    nc.vector.tensor_copy(v_all[:, ci0 * 256:(ci0 + 2) * 256], vps)

            # ---------------- attention (att^T = V^T @ P, channel-major) ----------------
            attps = [ps.tile([128, QS], F32, tag=f"attps{j}", bufs=1, name=f"attps{j}")
                     for j in range(2)]
            csum = ps.tile([2, QS], F32, tag="csum", bufs=1)
            for i in range(NBLK):
                sps = ps.tile([128, QS], F32, tag="work", name=f"sps{i}")
                nc.tensor.matmul(sps, kt[0][:, i * 128:(i + 1) * 128], qp[0],
                                 start=True, stop=False)
                nc.tensor.matmul(sps, kt[1][:, i * 128:(i + 1) * 128], qp[1],
                                 start=False, stop=True)
                pt = ptp.tile([128, QS], MMDT, tag="pt", name=f"pt{i}")
                nc.scalar.activation(pt, sps, AF.Exp)
                first, last = (i == 0), (i == NBLK - 1)
                for cb in range(2):
                    nc.tensor.matmul(attps[cb],
                                     v_all[:, i * 256 + cb * 128:i * 256 + (cb + 1) * 128],
                                     pt, start=first, stop=last)
                nc.tensor.matmul(csum, ones_c, pt, start=first, stop=last)

            # normalize: attnT = att_un^T * (1/colsum) broadcast over channels
            rr = sb.tile([1, QS], MMDT)
            with nc.allow_low_precision(reason="f32r/bf16 rounding of 1/denom"):
                nc.vector.reciprocal(rr, csum[0:1, :])
            rb = ps.tile([128, QS], F32, tag="work", name="rb")
            nc.tensor.matmul(rb, ones_r, rr, start=True, stop=True)
            attnT = [sb.tile([128, QS], MMDT, name=f"attnT{i}") for i in range(2)]
            for cb in range(2):
                att_s = sb.tile([128, QS], MMDT, name=f"att_s{cb}")
                nc.scalar.copy(att_s, attps[cb])
                nc.vector.tensor_mul(attnT[cb], att_s, rb)

            # ---------------- z = Wo @ attnT + bo', LayerNorm over channels ----------------
            zs = [sb.tile([128, QS], MMDT, name=f"zs{i}") for i in range(2)]
            zsq = [sb.tile([128, QS], MMDT, name=f"zsq{i}") for i in range(2)]
            for cb in range(2):
                zps = ps.tile([128, QS], F32, tag="work", name=f"zps{cb}")
                nc.tensor.matmul(zps, wsl(wo_t, 0, cb), attnT[0], start=True, stop=False)
                nc.tensor.matmul(zps, wsl(wo_t, 1, cb), attnT[1], start=False, stop=True)
                nc.scalar.activation(zs[cb], zps, AF.Identity, bias=bvec[cb][:, 1:2])
                nc.vector.tensor_mul(zsq[cb], zs[cb], zs[cb])

            szp = ps.tile([2, QS], F32, tag="work", name="szp")
            nc.tensor.matmul(szp, ones_c, zs[0], start=True, stop=False)
            nc.tensor.matmul(szp, ones_c, zs[1], start=False, stop=True)
            sqp = ps.tile([2, QS], F32, tag="work", name="sqp")
            nc.tensor.matmul(sqp, ones_c, zsq[0], start=True, stop=False)
            nc.tensor.matmul(sqp, ones_c, zsq[1], start=False, stop=True)

            neg_mean = sb.tile([1, QS], F32)
            nc.vector.tensor_scalar_mul(neg_mean, szp[0:1, :], -1.0 / C)
            m2 = sb.tile([1, QS], F32)
            nc.vector.tensor_mul(m2, neg_mean, neg_mean)
            var = sb.tile([1, QS], F32)
            nc.vector.scalar_tensor_tensor(var, sqp[0:1, :], 1.0 / C, m2,
                                           op0=ALU.mult, op1=ALU.subtract)
            lnv = sb.tile([1, QS], F32)
            nc.scalar.activation(lnv, var, AF.Ln, bias=epsv)
            rstd = sb.tile([1, QS], MMDT)
            nc.scalar.activation(rstd, lnv, AF.Exp, scale=-0.5)
            nmrs = sb.tile([1, QS], MMDT)
            nc.vector.tensor_mul(nmrs, neg_mean, rstd)

            rstd_b = ps.tile([128, QS], F32, tag="work", name="rstd_b")
            nc.tensor.matmul(rstd_b, ones_r, rstd, start=True, stop=True)
            nmrs_b = ps.tile([128, QS], F32, tag="work", name="nmrs_b")
            nc.tensor.matmul(nmrs_b, ones_r, nmrs, start=True, stop=True)

            zln = [sb.tile([128, QS], MMDT, name=f"zln{i}") for i in range(2)]
            for cb in range(2):
                zt = sb.tile([128, QS], MMDT, name=f"zt{cb}")
                nc.vector.tensor_mul(zt, zs[cb], rstd_b)
                nc.vector.tensor_add(zln[cb], zt, nmrs_b)

            # ---------------- MLP + residual ----------------
            hs = [sb.tile([128, QS], MMDT, name=f"hs{i}") for i in range(4)]
            for hb in range(4):
                hps = ps.tile([128, QS], F32, tag="work", name=f"hps{hb}")
                nc.tensor.matmul(hps, wsl(w1_t, 0, hb), zln[0], start=True, stop=False)
                nc.tensor.matmul(hps, wsl(w1_t, 1, hb), zln[1], start=False, stop=True)
                nc.scalar.activation(hs[hb], hps, AF.Gelu, bias=b1p[:, hb:hb + 1])

            for cb in range(2):
                tps2 = ps.tile([128, QS], F32, tag="work", name=f"tps2{cb}")
                for hb in range(4):
                    nc.tensor.matmul(tps2, w2_t[:, hb * 256 + cb * 128:hb * 256 + (cb + 1) * 128],
                                     hs[hb], start=(hb == 0), stop=(hb == 3))
                ot = sb.tile([128, QS], F32, name=f"ot{cb}")
                nc.vector.scalar_tensor_tensor(ot, tps2, bvec[cb][:, 2:3], xf[cb],
                                               op0=ALU.add, op1=ALU.add)
                nc.sync.dma_start(d_out[cb * 128:(cb + 1) * 128, :], ot)

    nc.compile()
    return nc


_NC = None


def _get_nc():
    global _NC
    if _NC is None:
        _NC = _build_nc()
    return _NC


def _pack_rows(a, nchunk):
    """(nchunk*128, W) -> (128, nchunk*W) with row-chunks side by side."""
    w = a.shape[1]
    out = np.empty((128, nchunk * w), a.dtype)
    for i in range(nchunk):
        out[:, i * w:(i + 1) * w] = a[i * 128:(i + 1) * 128, :]
    return out


def prep_in_maps(x, y, Wq, bq, Wk, bk, Wv, bv, Wo, bo, ln_w, ln_b, W1, b1, W2, b2):
    f = lambda a: np.asarray(a, dtype=np.float32)
    x, y = f(x), f(y)
    Wq, bq, Wk, Wv, bv, Wo, bo = f(Wq), f(bq), f(Wk), f(Wv), f(bv), f(Wo), f(bo)
    ln_w, ln_b, W1, b1, W2, b2 = f(ln_w), f(ln_b), f(W1), f(b1), f(W2), f(b2)

    mmnp = mybir.dt.np(MMDT)
    g = lambda a: np.ascontiguousarray(a).astype(mmnp)

    x_cm = np.ascontiguousarray(x.reshape(C, HW))
    y_cm = np.ascontiguousarray(y.reshape(C, NCTX))

    # host-side algebraic folds
    bo_p = (Wo.astype(np.float64) @ bv.astype(np.float64) + bo).astype(np.float32)
    b1_p = (W1.astype(np.float64) @ ln_b.astype(np.float64) + b1).astype(np.float32)
    W1p = (W1 * ln_w[None, :]).astype(np.float32)

    bvec = np.stack([bq / 16.0, bo_p, b2], axis=1).astype(np.float32)  # (256,3)

    common = {
        "y_mm": g(_pack_rows(y_cm, 2)),
        "wq_mm": g(_pack_rows(Wq.T, 2)),
        "wk_mm": g(_pack_rows(Wk.T, 2)),
        "wv_mm": g(_pack_rows(Wv.T, 2)),
        "wo_mm": g(_pack_rows(Wo.T, 2)),
        "w1_mm": g(_pack_rows(W1p.T, 2)),
        "w2_mm": g(_pack_rows(W2.T, 4)),
        "bvec": bvec,
        "b1p": np.ascontiguousarray(b1_p.reshape(4, 128).T),
        "ones_c": np.ones((128, 2), mmnp),
        "ones_r": np.ones((1, 128), mmnp),
    }
    in_maps = []
    for i in range(NCORES):
        m = dict(common)
        xs = np.ascontiguousarray(x_cm[:, i * QS:(i + 1) * QS])
        m["x_f32"] = xs
        m["x_mm"] = g(_pack_rows(xs, 2))
        in_maps.append(m)
    return in_maps


def kernel(**inputs):
    in_maps = prep_in_maps(**inputs)
    nc = _get_nc()
    res = bass_utils.run_bass_kernel_spmd(nc, in_maps, core_ids=list(range(NCORES)))
    t = np.concatenate([res.results[i]["out_sh"] for i in range(NCORES)], axis=1)
    return t.reshape(1, C, 64, 64)


# revision 10
# speedup vs baseline: 1.0224x; 1.0224x over previous
"""Trainium2 Bass kernel for nn_CMEncoder (cross-attention + LayerNorm2d + MLP block).

Strategy (8 NeuronCores, sequence-parallel over the HW=4096 query tokens):
  - Each core owns 512 query tokens; K/V over the full 4096-token context are
    computed redundantly on every core (no collectives needed).
  - Everything stays channel-major on chip ([feature partition, token free]).
  - Scores are computed transposed (S^T[n, q]) so P = exp(S^T) is the moving
    operand of the P@V matmuls (att^T = V^T @ P, N=512 full-rate); the softmax
    denominator comes from a cheap ones-stationary reduction matmul.
  - Host-side algebraic folds: bk dropped (softmax shift invariance), bv folded
    into the output-projection bias, the 1/sqrt(C) scale folded into the Q
    bias/scale, LayerNorm's affine folded into the MLP's first layer.
  - Matmuls run in float32r (full-rate fp32 on the PE array).
"""

import numpy as np
import concourse.bacc as bacc
import concourse.mybir as mybir
import concourse.tile as tile
from concourse import bass_utils
from concourse.hw_specs import get_activation_tables

F32 = mybir.dt.float32
F32R = mybir.dt.float32r
BF16 = mybir.dt.bfloat16
AF = mybir.ActivationFunctionType
ALU = mybir.AluOpType

MMDT = BF16      # matmul operand dtype (F32R or BF16)

C = 256          # channels
HW = 4096        # query tokens (64x64)
NCTX = 4096      # context tokens
HID = 512        # mlp hidden
NCORES = 8
QS = HW // NCORES   # 512 queries per core
NBLK = NCTX // 128  # 32 context chunks
EPS = 1e-6


def _build_nc():
    nc = bacc.Bacc("TRN2", target_bir_lowering=False)

    # --- DRAM I/O (weights pre-packed on host: row-chunks side by side) ---
    d_xmm = nc.dram_tensor("x_mm", (128, 2 * QS), MMDT, kind="ExternalInput")
    d_xf = nc.dram_tensor("x_f32", (C, QS), F32, kind="ExternalInput")
    d_y = nc.dram_tensor("y_mm", (128, 2 * NCTX), MMDT, kind="ExternalInput")
    d_wq = nc.dram_tensor("wq_mm", (128, 2 * C), MMDT, kind="ExternalInput")
    d_wk = nc.dram_tensor("wk_mm", (128, 2 * C), MMDT, kind="ExternalInput")
    d_wv = nc.dram_tensor("wv_mm", (128, 2 * C), MMDT, kind="ExternalInput")
    d_wo = nc.dram_tensor("wo_mm", (128, 2 * C), MMDT, kind="ExternalInput")
    d_w1 = nc.dram_tensor("w1_mm", (128, 2 * HID), MMDT, kind="ExternalInput")
    d_w2 = nc.dram_tensor("w2_mm", (128, 4 * C), MMDT, kind="ExternalInput")
    d_bv = nc.dram_tensor("bvec", (C, 3), F32, kind="ExternalInput")   # [bq/16, bo', b2]
    d_b1 = nc.dram_tensor("b1p", (128, 4), F32, kind="ExternalInput")
    d_oc = nc.dram_tensor("ones_c", (128, 2), MMDT, kind="ExternalInput")
    d_or = nc.dram_tensor("ones_r", (1, 128), MMDT, kind="ExternalInput")
    d_out = nc.dram_tensor("out_sh", (C, QS), F32, kind="ExternalOutput")

    tabs = list(get_activation_tables(nc.m.arch).keys())
    LNEXP_SET = tabs.index("natural_log_exp_and_others")

    with tile.TileContext(nc) as tc:
        # Pre-load the exp+ln activation table once so the auto-inserted loads
        # don't ping-pong between exp-only and ln-only sets mid-kernel.
        nc.scalar.add_instruction(mybir.InstLoadActFuncSet(
            name=nc.get_next_instruction_name(), ins=[], outs=[],
            act_func_set_id=LNEXP_SET))

        with (
            tc.tile_pool(name="sb", bufs=1) as sb,
            tc.tile_pool(name="pt_pool", bufs=3) as ptp,
            tc.tile_pool(name="ps", bufs=4, space="PSUM") as ps,
        ):
            # ---------------- input DMAs ----------------
            # compute-critical first; split issue across the Sync HW queue and
            # the GpSimd SW queue so descriptor generation isn't serialized.
            xmm = sb.tile([128, 2 * QS], MMDT)
            nc.sync.dma_start(xmm, d_xmm[:, :])
            wq_t = sb.tile([128, 2 * C], MMDT)
            nc.sync.dma_start(wq_t, d_wq[:, :])
            yt = [[sb.tile([128, NCTX // 2], MMDT, name=f"y{i}{h}") for h in range(2)]
                  for i in range(2)]
            for h in range(2):
                for i in range(2):
                    nc.sync.dma_start(
                        yt[i][h],
                        d_y[:, i * NCTX + h * (NCTX // 2):
                            i * NCTX + (h + 1) * (NCTX // 2)])

            wk_t = sb.tile([128, 2 * C], MMDT)
            nc.gpsimd.dma_start(wk_t, d_wk[:, :])
            wv_t = sb.tile([128, 2 * C], MMDT)
            nc.gpsimd.dma_start(wv_t, d_wv[:, :])
            bvec = [sb.tile([128, 3], F32, name=f"bvec{i}") for i in range(2)]
            for i in range(2):
                nc.gpsimd.dma_start(bvec[i], d_bv[i * 128:(i + 1) * 128, :])
            ones_c = sb.tile([128, 2], MMDT)
            nc.gpsimd.dma_start(ones_c, d_oc[:, :])
            ones_r = sb.tile([1, 128], MMDT)
            nc.gpsimd.dma_start(ones_r, d_or[:, :])
            wo_t = sb.tile([128, 2 * C], MMDT)
            nc.gpsimd.dma_start(wo_t, d_wo[:, :])
            w1_t = sb.tile([128, 2 * HID], MMDT)
            nc.gpsimd.dma_start(w1_t, d_w1[:, :])
            w2_t = sb.tile([128, 4 * C], MMDT)
            nc.gpsimd.dma_start(w2_t, d_w2[:, :])
            b1p = sb.tile([128, 4], F32)
            nc.gpsimd.dma_start(b1p, d_b1[:, :])
            xf = [sb.tile([128, QS], F32, name=f"xf{i}") for i in range(2)]
            for i in range(2):
                nc.gpsimd.dma_start(xf[i], d_xf[i * 128:(i + 1) * 128, :])

            epsv = sb.tile([1, 1], F32)
            nc.vector.memset(epsv, EPS)

            def wsl(t, cc, cb, w=128):
                # packed weight tile slice: row-chunk cc, col-chunk cb
                return t[:, cc * (t.shape[1] // 2) + cb * w:
                         cc * (t.shape[1] // 2) + (cb + 1) * w]

            # ---------------- Q' = (x^T Wq^T + bq)/16, channel-major ----------------
            qp = [sb.tile([128, QS], MMDT, name=f"qp{i}") for i in range(2)]
            for cb in range(2):
                qps = ps.tile([128, QS], F32, tag="work", name=f"qps{cb}")
                nc.tensor.matmul(qps, wsl(wq_t, 0, cb), xmm[:, 0:QS],
                                 start=True, stop=False)
                nc.tensor.matmul(qps, wsl(wq_t, 1, cb), xmm[:, QS:2 * QS],
                                 start=False, stop=True)
                nc.scalar.activation(qp[cb], qps, AF.Identity,
                                     bias=bvec[cb][:, 0:1], scale=1.0 / 16.0)

            # ---------------- K^T and V (token-major) ----------------
            kt = [sb.tile([128, NCTX], MMDT, name=f"kt{i}") for i in range(2)]
            v_all = sb.tile([128, NBLK * 256], MMDT)

            for nb in range(8):
                h = nb // 4
                col = (nb % 4) * 512
                for cb in range(2):
                    kps = ps.tile([128, 512], F32, tag="work", name=f"kps{cb}_{nb}")
                    nc.tensor.matmul(kps, wsl(wk_t, 0, cb),
                                     yt[0][h][:, col:col + 512], start=True, stop=False)
                    nc.tensor.matmul(kps, wsl(wk_t, 1, cb),
                                     yt[1][h][:, col:col + 512], start=False, stop=True)
                    nc.scalar.copy(kt[cb][:, nb * 512:(nb + 1) * 512], kps)
                for p2 in range(2):
                    vps = ps.tile([128, 512], F32, tag="work", name=f"vps{nb}_{p2}")
                    for k in range(2):
                        ci = nb * 4 + p2 * 2 + k   # context chunk index
                        c0 = (ci * 128) % 2048     # column within the y half-tile
                        nc.tensor.matmul(vps[:, k * 256:(k + 1) * 256],
                                         yt[0][h][:, c0:c0 + 128],
                                         wv_t[:, 0:256], start=True, stop=False)
                        nc.tensor.matmul(vps[:, k * 256:(k + 1) * 256],
                                         yt[1][h][:, c0:c0 + 128],
                                         wv_t[:, 256:512], start=False, stop=True)
                    ci0 = nb * 4 + p2 * 2
                    nc.vector.tensor_copy(v_all[:, ci0 * 256:(ci0 + 2) * 256], vps)

            # ---------------- attention (att^T = V^T @ P, channel-major) ----------------
            attps = [ps.tile([128, QS], F32, tag=f"attps{j}", bufs=1, name=f"attps{j}")
                     for j in range(2)]
            csum = ps.tile([2, QS], F32, tag="csum", bufs=1)
            for i in range(NBLK):
                sps = ps.tile([128, QS], F32, tag="work", name=f"sps{i}")
                nc.tensor.matmul(sps, kt[0][:, i * 128:(i + 1) * 128], qp[0],
                                 start=True, stop=False)
                nc.tensor.matmul(sps, kt[1][:, i * 128:(i + 1) * 128], qp[1],
                                 start=False, stop=True)
                pt = ptp.tile([128, QS], MMDT, tag="pt", name=f"pt{i}")
                nc.scalar.activation(pt, sps, AF.Exp)
                first, last = (i == 0), (i == NBLK - 1)
                for cb in range(2):
                    nc.tensor.matmul(attps[cb],
                                     v_all[:, i * 256 + cb * 128:i * 256 + (cb + 1) * 128],
                                     pt, start=first, stop=last)
                nc.tensor.matmul(csum, ones_c, pt, start=first, stop=last)

            # normalize: attnT = att_un^T * (1/colsum) broadcast over channels.
            # 1/x via exp(-ln(x)) on ACT: the DVE reciprocal on a single
            # partition is an iterative divide (~3.3us serial).
            lncs = sb.tile([1, QS], F32)
            nc.scalar.activation(lncs, csum[0:1, :], AF.Ln)
            rr = sb.tile([1, QS], MMDT)
            nc.scalar.activation(rr, lncs, AF.Exp, scale=-1.0)
            rb = ps.tile([128, QS], F32, tag="work", name="rb")
            nc.tensor.matmul(rb, ones_r, rr, start=True, stop=True)
            attnT = [sb.tile([128, QS], MMDT, name=f"attnT{i}") for i in range(2)]
            for cb in range(2):
                att_s = sb.tile([128, QS], MMDT, name=f"att_s{cb}")
                nc.scalar.copy(att_s, attps[cb])
                nc.vector.tensor_mul(attnT[cb], att_s, rb)

            # ---------------- z = Wo @ attnT + bo', LayerNorm over channels ----------------
            zs = [sb.tile([128, QS], MMDT, name=f"zs{i}") for i in range(2)]
            zsq = [sb.tile([128, QS], MMDT, name=f"zsq{i}") for i in range(2)]
            for cb in range(2):
                zps = ps.tile([128, QS], F32, tag="work", name=f"zps{cb}")
                nc.tensor.matmul(zps, wsl(wo_t, 0, cb), attnT[0], start=True, stop=False)
                nc.tensor.matmul(zps, wsl(wo_t, 1, cb), attnT[1], start=False, stop=True)
                nc.scalar.activation(zs[cb], zps, AF.Identity, bias=bvec[cb][:, 1:2])
                nc.vector.tensor_mul(zsq[cb], zs[cb], zs[cb])

            szp = ps.tile([2, QS], F32, tag="work", name="szp")
            nc.tensor.matmul(szp, ones_c, zs[0], start=True, stop=False)
            nc.tensor.matmul(szp, ones_c, zs[1], start=False, stop=True)
            sqp = ps.tile([2, QS], F32, tag="work", name="sqp")
            nc.tensor.matmul(sqp, ones_c, zsq[0], start=True, stop=False)
            nc.tensor.matmul(sqp, ones_c, zsq[1], start=False, stop=True)

            neg_mean = sb.tile([1, QS], F32)
            nc.vector.tensor_scalar_mul(neg_mean, szp[0:1, :], -1.0 / C)
            m2 = sb.tile([1, QS], F32)
            nc.vector.tensor_mul(m2, neg_mean, neg_mean)
            var = sb.tile([1, QS], F32)
            nc.vector.scalar_tensor_tensor(var, sqp[0:1, :], 1.0 / C, m2,
                                           op0=ALU.mult, op1=ALU.subtract)
            lnv = sb.tile([1, QS], F32)
            nc.scalar.activation(lnv, var, AF.Ln, bias=epsv)
            rstd = sb.tile([1, QS], MMDT)
            nc.scalar.activation(rstd, lnv, AF.Exp, scale=-0.5)
            nmrs = sb.tile([1, QS], MMDT)
            nc.vector.tensor_mul(nmrs, neg_mean, rstd)

            rstd_b = ps.tile([128, QS], F32, tag="work", name="rstd_b")
            nc.tensor.matmul(rstd_b, ones_r, rstd, start=True, stop=True)
            nmrs_b = ps.tile([128, QS], F32, tag="work", name="nmrs_b")
            nc.tensor.matmul(nmrs_b, ones_r, nmrs, start=True, stop=True)

            zln = [sb.tile([128, QS], MMDT, name=f"zln{i}") for i in range(2)]
            for cb in range(2):
                zt = sb.tile([128, QS], MMDT, name=f"zt{cb}")
                nc.vector.tensor_mul(zt, zs[cb], rstd_b)
                nc.vector.tensor_add(zln[cb], zt, nmrs_b)

            # ---------------- MLP + residual ----------------
            hs = [sb.tile([128, QS], MMDT, name=f"hs{i}") for i in range(4)]
            for hb in range(4):
                hps = ps.tile([128, QS], F32, tag="work", name=f"hps{hb}")
                nc.tensor.matmul(hps, wsl(w1_t, 0, hb), zln[0], start=True, stop=False)
                nc.tensor.matmul(hps, wsl(w1_t, 1, hb), zln[1], start=False, stop=True)
                nc.scalar.activation(hs[hb], hps, AF.Gelu, bias=b1p[:, hb:hb + 1])

            for cb in range(2):
                tps2 = ps.tile([128, QS], F32, tag="work", name=f"tps2{cb}")
                for hb in range(4):
                    nc.tensor.matmul(tps2, w2_t[:, hb * 256 + cb * 128:hb * 256 + (cb + 1) * 128],
                                     hs[hb], start=(hb == 0), stop=(hb == 3))
                ot = sb.tile([128, QS], F32, name=f"ot{cb}")
                nc.vector.scalar_tensor_tensor(ot, tps2, bvec[cb][:, 2:3], xf[cb],
                                               op0=ALU.add, op1=ALU.add)
                nc.sync.dma_start(d_out[cb * 128:(cb + 1) * 128, :], ot)

    nc.compile()
    return nc


_NC = None


def _get_nc():
    global _NC
    if _NC is None:
        _NC = _build_nc()
    return _NC


def _pack_rows(a, nchunk):
    """(nchunk*128, W) -> (128, nchunk*W) with row-chunks side by side."""
    w = a.shape[1]
    out = np.empty((128, nchunk * w), a.dtype)
    for i in range(nchunk):
        out[:, i * w:(i + 1) * w] = a[i * 128:(i + 1) * 128, :]
    return out


def prep_in_maps(x, y, Wq, bq, Wk, bk, Wv, bv, Wo, bo, ln_w, ln_b, W1, b1, W2, b2):
    f = lambda a: np.asarray(a, dtype=np.float32)
    x, y = f(x), f(y)
    Wq, bq, Wk, Wv, bv, Wo, bo = f(Wq), f(bq), f(Wk), f(Wv), f(bv), f(Wo), f(bo)
    ln_w, ln_b, W1, b1, W2, b2 = f(ln_w), f(ln_b), f(W1), f(b1), f(W2), f(b2)

    mmnp = mybir.dt.np(MMDT)
    g = lambda a: np.ascontiguousarray(a).astype(mmnp)

    x_cm = np.ascontiguousarray(x.reshape(C, HW))
    y_cm = np.ascontiguousarray(y.reshape(C, NCTX))

    # host-side algebraic folds
    bo_p = (Wo.astype(np.float64) @ bv.astype(np.float64) + bo).astype(np.float32)
    b1_p = (W1.astype(np.float64) @ ln_b.astype(np.float64) + b1).astype(np.float32)
    W1p = (W1 * ln_w[None, :]).astype(np.float32)

    bvec = np.stack([bq / 16.0, bo_p, b2], axis=1).astype(np.float32)  # (256,3)

    common = {
        "y_mm": g(_pack_rows(y_cm, 2)),
        "wq_mm": g(_pack_rows(Wq.T, 2)),
        "wk_mm": g(_pack_rows(Wk.T, 2)),
        "wv_mm": g(_pack_rows(Wv.T, 2)),
        "wo_mm": g(_pack_rows(Wo.T, 2)),
        "w1_mm": g(_pack_rows(W1p.T, 2)),
        "w2_mm": g(_pack_rows(W2.T, 4)),
        "bvec": bvec,
        "b1p": np.ascontiguousarray(b1_p.reshape(4, 128).T),
        "ones_c": np.ones((128, 2), mmnp),
        "ones_r": np.ones((1, 128), mmnp),
    }
    in_maps = []
    for i in range(NCORES):
        m = dict(common)
        xs = np.ascontiguousarray(x_cm[:, i * QS:(i + 1) * QS])
        m["x_f32"] = xs
        m["x_mm"] = g(_pack_rows(xs, 2))
        in_maps.append(m)
    return in_maps


def kernel(**inputs):
    in_maps = prep_in_maps(**inputs)
    nc = _get_nc()
    res = bass_utils.run_bass_kernel_spmd(nc, in_maps, core_ids=list(range(NCORES)))
    t = np.concatenate([res.results[i]["out_sh"] for i in range(NCORES)], axis=1)
    return t.reshape(1, C, 64, 64)
